# revision 1
# baseline (speedup 1.0000x reference)
"""NT-Xent loss (SimCLR) on 8 Trainium2 NeuronCores.

Contract: kernel(z_i, z_j) -> np.float32 scalar loss, matching the
reference NT-Xent (temperature 0.5). Inputs are the full [4096, 128]
fp32 projection batches; sharding happens inside.

Strategy (per core c of 8):
  - rows of the 8192x8192 sim matrix are sharded: core c owns rows
    [c*1024, (c+1)*1024).
  - every core redundantly normalizes + transposes the full z
    (concat of z_i, z_j) into zhatT [128(D), 8192] bf16 on-chip; that is
    far cheaper than communicating it.
  - all SBUF loads use a per-partition-contiguous layout (partition p
    holds rows p*64..p*64+63 of z); this permutes rows/columns of the
    sim matrix, which is irrelevant because every result is summed.
  - row norms are computed in fp32 (scalar_tensor_tensor fused
    square+reduce); 1/sqrt via bit-trick seed + 2 Newton steps on the
    vector engine (keeps ScalarE on a single Exp table set).
  - both normalizations fuse into per-partition vector-engine scales in
    the natural layout (rows live on partitions there): slab rows are
    pre-scaled by 2/||row||, columns by 1/||row||, each fused with the
    fp32->bf16 cast; the PE then only runs plain bf16 transposes and
    bf16 sim matmuls, and the PSUM logits come out fully scaled.
  - exp + row-sum are fused in one ScalarE pass (scale=1) via accum_out
    over 2048-wide PSUM tiles (4 banks), double buffered; prep and main
    PSUM tiles share one pool with emission interleaved to match the
    allocator's in-order slot reuse.
  - the diagonal (masked with -inf in the reference) contributes exactly
    exp(2) to each raw row-sum; it is subtracted before the final log.
  - the final per-row log uses an exponent-split + atanh-series
    polynomial evaluated on the vector engine (the Ln activation table
    is not loadable in this runtime).
  - positives are computed from the raw fp32 slab/partner rows (per-core
    inputs), off the critical path.
  - each core writes [128, 16]: cols 0:8 lse per slab row, 8:16 pos per
    slab row. The host sums (lse - pos) over all cores / 8192.
"""

import os
import sys

if "/opt/trn_rl_repo" not in sys.path:
    sys.path.insert(0, "/opt/trn_rl_repo")

import numpy as np

import concourse.bacc as bacc
import concourse.mybir as mybir
import concourse.tile as tile
from concourse.bass_utils import run_bass_kernel_spmd

B = 4096
D = 128
N = 2 * B  # 8192 rows of the sim matrix
CORES = 8
SLAB = N // CORES  # 1024 rows per core
NT = N // 128  # 64 partition-tiles of z
ST = SLAB // 128  # 8 slab tiles
GROUPS = 8
GT = NT // GROUPS
NB = 4  # main-loop column blocks of 2048
EXP2 = float(np.exp(2.0))
LN2 = float(np.log(2.0))
MAGIC = 0x5F3759DF

f32 = mybir.dt.float32
bf16 = mybir.dt.bfloat16
u32 = mybir.dt.uint32


def build_nc():
    nc = bacc.Bacc("TRN2", target_bir_lowering=False, debug=False, num_devices=CORES)
    z = nc.dram_tensor("z", [N, D], f32, kind="ExternalInput").ap()
    zs = nc.dram_tensor("zs", [SLAB, D], f32, kind="ExternalInput").ap()
    zp = nc.dram_tensor("zp", [SLAB, D], f32, kind="ExternalInput").ap()
    eye = nc.dram_tensor("eye", [128, 128], f32, kind="ExternalInput").ap()
    out = nc.dram_tensor("out", [128, 16], f32, kind="ExternalOutput").ap()

    AF = mybir.ActivationFunctionType
    OP = mybir.AluOpType

    with tile.TileContext(nc) as tc:
        with (
            tc.tile_pool(name="big", bufs=1) as big,
            tc.tile_pool(name="stats", bufs=1) as stats,
            tc.tile_pool(name="work", bufs=3) as work,
            tc.tile_pool(name="mm_ps", bufs=2, space="PSUM") as mm_ps_pool,
        ):
            # ---- persistent SBUF tensors ----
            zn = big.tile([128, N], f32, tag="zn")  # partition p: rows p*64+t
            znhat = big.tile([128, N], bf16, tag="znhat")  # normalized z, bf16
            zsb = big.tile([128, SLAB], bf16, tag="zsb")  # raw slab, bf16
            zhatT = big.tile([128, N], bf16, tag="zhatT")  # normalized z, transposed
            slabT = big.tile([128, SLAB], bf16, tag="slabT")  # raw slab, transposed
            zs_n = big.tile([128, SLAB], f32, tag="zs_n")
            zp_n = big.tile([128, SLAB], f32, tag="zp_n")
            eye_t = stats.tile([128, 128], f32, tag="eye")
            eye_b = stats.tile([128, 128], bf16, tag="eye_b")
            s_full = stats.tile([128, NT], f32, tag="s_full")  # row sumsq of z
            invn = stats.tile([128, NT], f32, tag="invn")  # 1/||z_r||
            s_s = stats.tile([128, ST], f32, tag="s_s")
            s_p = stats.tile([128, ST], f32, tag="s_p")
            sc2 = stats.tile([128, ST], f32, tag="sc2")  # 2/||z_slab_r||
            invn_p = stats.tile([128, ST], f32, tag="invn_p")
            posdot = stats.tile([128, ST], f32, tag="posdot")
            post1 = stats.tile([128, ST], f32, tag="post1")
            ra = stats.tile([128, NT], f32, tag="ra")  # rsqrt scratch
            rb = stats.tile([128, NT], f32, tag="rb")
            rh = stats.tile([128, NT], f32, tag="rh")
            rowparts = stats.tile([128, ST * NB], f32, tag="rowparts")
            rowsums = stats.tile([128, ST], f32, tag="rowsums")
            outbuf = stats.tile([128, 16], f32, tag="outbuf")
            waste = stats.tile([128, 2048], f32, tag="waste")  # exp values, unread
            sq_scr = stats.tile([128, 128], f32, tag="sq_scr")  # STT out, unread
            sq_scr2 = stats.tile([128, 128], f32, tag="sq_scr2")  # ACT square out
            # poly-ln scratch, all [128, ST]
            lx = stats.tile([128, ST], f32, tag="lx")
            lu = stats.tile([128, ST], u32, tag="lu")
            le = stats.tile([128, ST], f32, tag="le")
            lm = stats.tile([128, ST], u32, tag="lm")
            lnum = stats.tile([128, ST], f32, tag="lnum")
            lden = stats.tile([128, ST], f32, tag="lden")
            lt = stats.tile([128, ST], f32, tag="lt")
            lw = stats.tile([128, ST], f32, tag="lw")
            lp = stats.tile([128, ST], f32, tag="lp")

            def sumsq(a, b, acc):
                # acc[p] = sum_f a[p,f]*b[p,f]; out tile is scratch
                nc.vector.scalar_tensor_tensor(
                    sq_scr[:], a, 1.0, b, OP.mult, OP.mult, accum_out=acc
                )

            def sumsq_act(a, acc):
                nc.scalar.activation(
                    sq_scr2[:], a, AF.Square, bias=0.0, scale=1.0, accum_out=acc
                )

            def rsqrt(s_ap, out_ap, c):
                # out = 1/sqrt(s): quake seed + 2 Newton steps, all on DVE.
                # The MAGIC - (bits>>1) subtraction runs in f32 value domain
                # (uint add/sub wraparound is unreliable here); the ~2^-18
                # relative rounding this adds is irrelevant for a seed.
                bits = s_ap.bitcast(u32)
                sa = ra[:, 0:c]
                sb = rb[:, 0:c]
                sh = rh[:, 0:c]
                sa_u = sa.bitcast(u32)
                nc.vector.tensor_scalar(sa_u, bits, 1, None, OP.logical_shift_right)
                nc.vector.tensor_copy(sb, sa_u)  # u32 -> f32 value
                nc.vector.tensor_scalar(
                    sb, sb, float(MAGIC), -1.0, OP.subtract, OP.mult
                )  # MAGIC - v
                nc.vector.tensor_copy(sa_u, sb)  # f32 value -> u32 bits
                nc.vector.tensor_mul(sh, sa, sa)
                nc.vector.tensor_mul(sh, sh, s_ap)
                nc.vector.tensor_scalar(sh, sh, -0.5, 1.5, OP.mult, OP.add)
                nc.vector.tensor_mul(sb, sa, sh)
                nc.vector.tensor_mul(sh, sb, sb)
                nc.vector.tensor_mul(sh, sh, s_ap)
                nc.vector.tensor_scalar(sh, sh, -0.5, 1.5, OP.mult, OP.add)
                nc.vector.tensor_mul(out_ap, sb, sh)

            def rsqrt1(s_ap, out_ap, c):
                # single-Newton variant (rel err ~1.7e-3 -> ~-4e-4 bias; fine
                # for column scales feeding exp)
                bits = s_ap.bitcast(u32)
                sa = ra[:, 0:c]
                sb = rb[:, 0:c]
                sh = rh[:, 0:c]
                sa_u = sa.bitcast(u32)
                nc.vector.tensor_scalar(sa_u, bits, 1, None, OP.logical_shift_right)
                nc.vector.tensor_copy(sb, sa_u)
                nc.vector.tensor_scalar(
                    sb, sb, float(MAGIC), -1.0, OP.subtract, OP.mult
                )
                nc.vector.tensor_copy(sa_u, sb)
                nc.vector.tensor_mul(sh, sa, sa)
                nc.vector.tensor_mul(sh, sh, s_ap)
                nc.vector.tensor_scalar(sh, sh, -0.5, 1.5, OP.mult, OP.add)
                nc.vector.tensor_mul(out_ap, sa, sh)

            nc.sync.dma_start(eye_t[:], eye[:])
            nc.vector.tensor_copy(eye_b[:], eye_t[:])

            # ---- loads: per-partition contiguous (partition p <- rows p*K+i) ----
            # Order matters: the slab (zs) gates the whole main loop, then the
            # first two z chunks (first column block), then zp (positives).
            zv = z.rearrange("(p n) d -> p n d", p=128)  # [128, 64, 128]
            zsv = zs.rearrange("(p n) d -> p n d", p=128)
            zpv = zp.rearrange("(p n) d -> p n d", p=128)
            nc.sync.dma_start(zs_n[:, 0 : SLAB // 2], zsv[:, 0 : ST // 2, :])
            nc.sync.dma_start(zs_n[:, SLAB // 2 :], zsv[:, ST // 2 :, :])

            def load_chunk(g):
                nc.sync.dma_start(
                    zn[:, g * GT * 128 : (g + 1) * GT * 128],
                    zv[:, g * GT : (g + 1) * GT, :],
                )

            load_chunk(0)
            load_chunk(1)
            nc.sync.dma_start(zp_n[:], zpv[:])
            for g in range(2, GROUPS):
                load_chunk(g)

            # ---- slab: sumsq -> sc2 (needed by main exp), raw transpose ----
            for t in range(ST):
                zst = zs_n[:, t * 128 : (t + 1) * 128]
                sumsq(zst, zst, s_s[:, t : t + 1])
            rsqrt1(s_s[:], sc2[:], ST)
            nc.vector.tensor_scalar(sc2[:], sc2[:], 2.0, None, OP.mult)

            # slab scaled transpose -> slabT bf16: pre-scaling rows by
            # 2/||row|| here makes the PSUM logits fully scaled, so the exp
            # runs with a constant scale.
            for t in range(ST):
                nc.vector.tensor_scalar_mul(
                    zsb[:, t * 128 : (t + 1) * 128],
                    zs_n[:, t * 128 : (t + 1) * 128],
                    sc2[:, t : t + 1],
                )
            ppsb = mm_ps_pool.tile([128, 2048], f32, tag="mm")
            ppsb_b = ppsb[:, 0:1024].bitcast(bf16)[:, 0:1024]
            for t in range(ST):
                nc.tensor.transpose(
                    ppsb_b[:, t * 128 : (t + 1) * 128],
                    zsb[:, t * 128 : (t + 1) * 128],
                    eye_b[:],
                )
            nc.vector.tensor_copy(slabT[:], ppsb_b[:])

            # ---- full-z prep: sumsq -> invn -> bf16 cast -> diag matmul ----
            for g in range(GROUPS):
                lo, hi = g * GT, (g + 1) * GT
                for i in range(GT):
                    t = g * GT + i
                    znt = zn[:, t * 128 : (t + 1) * 128]
                    if t % 2 == 1 and g < 4:
                        sumsq_act(znt, s_full[:, t : t + 1])
                    else:
                        sumsq(znt, znt, s_full[:, t : t + 1])
                def scale_tiles(a, b):
                    for t in range(a, b):
                        nc.vector.tensor_scalar_mul(
                            znhat[:, t * 128 : (t + 1) * 128],
                            zn[:, t * 128 : (t + 1) * 128],
                            invn[:, t : t + 1],
                        )

                if g < 4:
                    rsqrt1(s_full[:, lo:hi], invn[:, lo:hi], GT)
                    scale_tiles(lo, hi)
                elif g == GROUPS - 1:
                    rsqrt1(
                        s_full[:, 4 * GT : NT], invn[:, 4 * GT : NT], NT - 4 * GT
                    )
                    scale_tiles(4 * GT, NT)
            # ---- transpose blocks + main loop, emission-interleaved so the
            # shared PSUM pool's in-order slot allocator never makes a main
            # tile wait on a far-future prep block (or vice versa) ----
            def prep_block(blk):
                pps = mm_ps_pool.tile([128, 2048], f32, tag="mm")
                ppsb16 = pps[:].bitcast(bf16)[:, 0:2048]
                for j in range(16):
                    t = blk * 16 + j
                    nc.tensor.transpose(
                        ppsb16[:, j * 128 : (j + 1) * 128],
                        znhat[:, t * 128 : (t + 1) * 128],
                        eye_b[:],
                    )
                if blk < 1:
                    nc.scalar.copy(zhatT[:, blk * 2048 : (blk + 1) * 2048], ppsb16)
                else:
                    nc.vector.tensor_copy(
                        zhatT[:, blk * 2048 : (blk + 1) * 2048], ppsb16
                    )

            def main_tile(nb, m):
                ps = mm_ps_pool.tile([128, 2048], f32, tag="mm")
                for h in range(4):
                    col = nb * 2048 + h * 512
                    nc.tensor.matmul(
                        ps[:, h * 512 : (h + 1) * 512],
                        lhsT=slabT[:, m * 128 : (m + 1) * 128],
                        rhs=zhatT[:, col : col + 512],
                        start=True,
                        stop=True,
                    )
                nc.scalar.activation(
                    waste[:],
                    ps[:],
                    AF.Exp,
                    bias=0.0,
                    scale=1.0,
                    accum_out=rowparts[:, m * NB + nb : m * NB + nb + 1],
                )

            prep_block(0)
            main_tile(0, 0)
            main_tile(0, 1)
            main_tile(0, 2)
            main_tile(0, 3)
            prep_block(1)
            main_tile(0, 4)
            main_tile(0, 5)
            main_tile(0, 6)
            main_tile(0, 7)
            prep_block(2)
            for m in range(4):
                main_tile(1, m)
            prep_block(3)
            for m in range(4, ST):
                main_tile(1, m)
            for m in range(ST):
                main_tile(2, m)

            for m in range(ST):
                main_tile(3, m)


            # ---- positives (off critical path) ----
            for t in range(ST):
                zst = zs_n[:, t * 128 : (t + 1) * 128]
                zpt = zp_n[:, t * 128 : (t + 1) * 128]
                sumsq(zpt, zpt, s_p[:, t : t + 1])
                sumsq(zst, zpt, posdot[:, t : t + 1])
            rsqrt1(s_p[:], invn_p[:], ST)
            # pos = posdot * (2*invn_s) * invn_p
            nc.vector.tensor_mul(post1[:], posdot[:], sc2[:])
            nc.vector.tensor_mul(outbuf[:, 8:16], post1[:], invn_p[:])

            # ---- epilogue: lse = log(rowsum - e^2) via exponent+poly ----
            nc.vector.tensor_reduce(
                rowsums[:],
                rowparts[:].rearrange("p (m n) -> p m n", m=ST),
                axis=mybir.AxisListType.X,
                op=OP.add,
            )
            nc.vector.tensor_scalar(lx[:], rowsums[:], EXP2, None, OP.subtract)
            bits = lx[:].bitcast(u32)
            nc.vector.tensor_scalar(lu[:], bits, 23, None, OP.logical_shift_right)
            nc.vector.tensor_copy(le[:], lu[:])  # uint -> f32 convert
            nc.vector.tensor_scalar(
                lm[:], bits, 0x007FFFFF, 0x3F800000, OP.bitwise_and, OP.bitwise_or
            )
            mf = lm[:].bitcast(f32)
            nc.vector.tensor_scalar(lnum[:], mf, 1.0, None, OP.subtract)
            nc.vector.tensor_scalar(lden[:], mf, 1.0, None, OP.add)
            nc.vector.reciprocal(lden[:], lden[:])
            nc.vector.tensor_mul(lt[:], lnum[:], lden[:])
            nc.vector.tensor_mul(lw[:], lt[:], lt[:])
            nc.vector.tensor_scalar(lp[:], lw[:], 2.0 / 9.0, 2.0 / 7.0, OP.mult, OP.add)
            nc.vector.tensor_mul(lp[:], lp[:], lw[:])
            nc.vector.tensor_scalar(lp[:], lp[:], 2.0 / 5.0, None, OP.add)
            nc.vector.tensor_mul(lp[:], lp[:], lw[:])
            nc.vector.tensor_scalar(lp[:], lp[:], 2.0 / 3.0, None, OP.add)
            nc.vector.tensor_mul(lp[:], lp[:], lw[:])
            nc.vector.tensor_scalar(lp[:], lp[:], 2.0, None, OP.add)
            nc.vector.tensor_mul(lp[:], lp[:], lt[:])  # ln(m)
            nc.vector.tensor_scalar(le[:], le[:], 127.0, None, OP.subtract)
            nc.vector.scalar_tensor_tensor(
                outbuf[:, 0:8], le[:], LN2, lp[:], OP.mult, OP.add
            )
            nc.sync.dma_start(out[:], outbuf[:])

    nc.compile()
    return nc


_NC_CACHE = {}


def _get_nc():
    if "nc" not in _NC_CACHE:
        _NC_CACHE["nc"] = build_nc()
    return _NC_CACHE["nc"]


def kernel(z_i, z_j):
    z_i = np.asarray(z_i, dtype=np.float32)
    z_j = np.asarray(z_j, dtype=np.float32)
    z = np.ascontiguousarray(np.concatenate([z_i, z_j], axis=0))
    eye = np.eye(128, dtype=np.float32)
    in_maps = []
    for c in range(CORES):
        r0 = c * SLAB
        p0 = (r0 + B) % N
        in_maps.append(
            {
                "z": z,
                "zs": np.ascontiguousarray(z[r0 : r0 + SLAB]),
                "zp": np.ascontiguousarray(z[p0 : p0 + SLAB]),
                "eye": eye,
            }
        )
    nc = _get_nc()
    kwargs = {}
    tdir = os.environ.get("NTX_TRACE_DIR")
    if tdir:
        kwargs = {"trace": True, "tmpdir": tdir, "trace_cores": [0]}
    res = run_bass_kernel_spmd(nc, in_maps, core_ids=list(range(CORES)), **kwargs)
    if tdir:
        _NC_CACHE["last_results"] = res
    tot = 0.0
    for c in range(CORES):
        o = res.results[c]["out"].astype(np.float64)
        tot += o[:, 0:8].sum() - o[:, 8:16].sum()
    return np.float32(tot / N)



# revision 2
# speedup vs baseline: 1.4720x; 1.4720x over previous
"""NT-Xent loss (SimCLR) on 8 Trainium2 NeuronCores — V2.

Contract: kernel(z_i, z_j) -> np.float32 scalar loss matching the
reference NT-Xent (temperature 0.5).

Per-core strategy (core c of 8 owns rows [c*1024, (c+1)*1024) of the
8192 x 8192 sim matrix):
  - Host prep ("all-gather the normalized projections" per the sharding
    hint): normalize z rows in fp32, quantize to fp8e4, and ship the
    transposed layout zT [128(D), 8192] that the PE needs — one shared
    array for every core.  Positives and the final log/mean run on host
    (O(N*D) / O(N) work); the O(N^2 * D) matmul + 64M exps stay on
    device.
  - PE: per (m, nb) tile, 4 fp8 matmuls [K=128] x [128, 512] produce a
    [128 slab rows, 2048 cols] fp32 PSUM tile (1 cycle/row at full
    clock; fp8 DoubleRow measured 2x SLOWER than this on HW, so plain
    matmuls are used).
  - The 8.4M exps per core are split across two engines working from a
    shared 2-buffer PSUM ring (8 banks):
      * ACT tiles: exp(2x) via activation scale=2.0 with accum_out ->
        per-row partial sums (1 elem/cycle/lane @1.2GHz).
      * DVE tiles: Schraudolph integer exp2 - pass1 builds bf16 bit
        patterns round((2x*log2e + 127)*128 - c) as i16, pass2
        tensor_reduce sums them as bf16.  Bias constant calibrated so
        the mean ratio to exp() is ~1; residual noise is far inside the
        2e-2 tolerance.
  - Device output: rowparts [128, 32] fp32 per core (one partial row
    sum per (m, nb)).  Host: rowsum per row, subtract the exact
    quantized diagonal exp(2*||q8(zhat_r)||^2), lse = log(.), pos from
    fp32 zhat, loss = mean(lse - pos).
"""

import os
import sys

if "/opt/trn_rl_repo" not in sys.path:
    sys.path.insert(0, "/opt/trn_rl_repo")

import numpy as np
import ml_dtypes

import concourse.bacc as bacc
import concourse.mybir as mybir
import concourse.tile as tile
from concourse.bass_utils import run_bass_kernel_spmd

B = 4096
D = 128
N = 2 * B  # 8192
CORES = 8
SLAB = N // CORES  # 1024 rows per core
MT = SLAB // 128  # 8 m-tiles
NB = N // 2048  # 4 column blocks
EPS = 1e-12

f32 = mybir.dt.float32
bf16 = mybir.dt.bfloat16
fp8 = mybir.dt.float8e4
i16 = mybir.dt.int16

AF = mybir.ActivationFunctionType
OP = mybir.AluOpType

LOG2E = float(np.log2(np.e))
# pass1: bits = round((2x)*log2e*128 + (127*128 - C)) ; bitcast bf16 = exp(2x)
A1 = 2.0 * 128.0 * LOG2E
C_CORR = 0.0574 * 128.0 - 1.204  # empirically debiased (exp1: -0.65% -> ~0)
B1 = 127.0 * 128.0 - C_CORR

# ACT:DVE tile split over the 32 (m, nb) tiles, interleaved evenly.
DVE_TILES = 10


def _consumer_pattern():
    """Return list of 32 bools: True = DVE tile, evenly spread."""
    pat = [False] * 32
    step = 32 / DVE_TILES
    for i in range(DVE_TILES):
        pat[min(31, int(round(i * step + step / 2)))] = True
    # ensure exactly DVE_TILES
    while sum(pat) < DVE_TILES:
        for i in range(32):
            if not pat[i]:
                pat[i] = True
                break
    return pat


def build_nc():
    nc = bacc.Bacc("TRN2", target_bir_lowering=False, debug=False, num_devices=CORES)
    zT = nc.dram_tensor("zT", [128, N], fp8, kind="ExternalInput").ap()
    slab_lo = nc.dram_tensor("slab_lo", [1, 1], f32, kind="ExternalInput").ap()
    out = nc.dram_tensor("out", [128, 32], f32, kind="ExternalOutput").ap()

    pat = _consumer_pattern()

    with tile.TileContext(nc) as tc:
        with (
            tc.tile_pool(name="big", bufs=1) as big,
            tc.tile_pool(name="ring", bufs=2, space="PSUM") as ring,
            tc.tile_pool(name="ebp", bufs=2) as ebp,
        ):
            zt = big.tile([128, N], fp8, tag="zt")
            rowparts = big.tile([128, 32], f32, tag="rowparts")
            wasteA = big.tile([128, 2048], f32, tag="wasteA")
            dummy = big.tile([128, 1], f32, tag="dummy")

            # warm the exp table set during the DMA phase
            nc.vector.memset(dummy[:], 0.0)
            nc.scalar.activation(
                dummy[:], dummy[:], AF.Exp, bias=0.0, scale=1.0
            )

            # slab columns first (stationary source), then the rest
            # slab for core c lives at columns [c*1024, (c+1)*1024).
            # We don't know c at build time (SPMD), but the DMA order only
            # affects startup by <1us; just load block 0 first and go in
            # order.  (The first tiles consume block 0 as moving side and
            # the slab as stationary; worst case the stationary arrives
            # with block 3 for core 7 — so issue a dedicated slab DMA
            # first using the per-core slab_base input trick is not
            # available; instead every core loads its own slab range
            # first via a per-core input alias.)
            # Simplest robust order: 4 block DMAs, block 0 first.
            for nb in range(NB):
                nc.sync.dma_start(
                    zt[:, nb * 2048 : (nb + 1) * 2048],
                    zT[:, nb * 2048 : (nb + 1) * 2048],
                )

            def act_tile(m, nb, ps):
                nc.scalar.activation(
                    wasteA[:],
                    ps[:],
                    AF.Exp,
                    bias=0.0,
                    scale=2.0,
                    accum_out=rowparts[:, m * NB + nb : m * NB + nb + 1],
                )

            def dve_tile(m, nb, ps):
                eb = ebp.tile([128, 2048], i16, tag="eb")
                nc.vector.tensor_scalar(eb[:], ps[:], A1, B1, OP.mult, OP.add)
                nc.vector.tensor_reduce(
                    rowparts[:, m * NB + nb : m * NB + nb + 1],
                    eb[:].bitcast(bf16),
                    axis=mybir.AxisListType.X,
                    op=OP.add,
                )

            # slab stationary slices come from zt at per-core offset.
            # SPMD trick: we cannot vary the AP per core, so instead the
            # host passes the slab as a SEPARATE per-core input region by
            # overwriting columns?  No — keep it simple: each core gets
            # its own copy of zT with its slab pre-swapped to the front
            # 1024 columns (host-side cheap).  Stationary = zt[:, 0:1024],
            # moving uses the ORIGINAL column order via a second tensor?
            # That would double DMA.  Instead: host sends zTc where
            # columns are ROTATED so the slab is first: cols' = concat(
            # slab_cols, other_cols).  Row sums are over all columns —
            # permutation-invariant.  Column index j in zTc maps to
            # global column (c*1024 + j) mod 8192, which only matters for
            # the host-side diag correction (row r appears as column
            # (r - c*1024) mod 8192 — never needed on device).
            idx = 0
            for nb in range(NB):
                for m in range(MT):
                    ps = ring.tile([128, 2048], f32, tag="mm")
                    for h in range(4):
                        nc.tensor.matmul(
                            ps[:, h * 512 : (h + 1) * 512],
                            lhsT=zt[:, m * 128 : (m + 1) * 128],
                            rhs=zt[:, nb * 2048 + h * 512 : nb * 2048 + (h + 1) * 512],
                            start=True,
                            stop=True,
                        )
                    if pat[idx]:
                        dve_tile(m, nb, ps)
                    else:
                        act_tile(m, nb, ps)
                    idx += 1

            nc.sync.dma_start(out[:], rowparts[:])

    nc.compile()
    return nc


_NC_CACHE = {}


def _get_nc():
    if "nc" not in _NC_CACHE:
        _NC_CACHE["nc"] = build_nc()
    return _NC_CACHE["nc"]


def kernel(z_i, z_j):
    z_i = np.asarray(z_i, dtype=np.float32)
    z_j = np.asarray(z_j, dtype=np.float32)
    z = np.concatenate([z_i, z_j], axis=0)  # [N, D]
    norm = np.sqrt((z * z).sum(axis=1, keepdims=True))
    zhat = z / np.maximum(norm, EPS)  # fp32 normalized, matches reference

    zq = zhat.astype(ml_dtypes.float8_e4m3)  # TRN fp8e4-compatible (max 240)
    zqT = np.ascontiguousarray(zq.T)  # [128, 8192]

    dummy = np.zeros((1, 1), dtype=np.float32)
    in_maps = []
    for c in range(CORES):
        # rotate columns so this core's slab is first (stationary region)
        rot = np.roll(zqT, -c * SLAB, axis=1)
        in_maps.append({"zT": np.ascontiguousarray(rot), "slab_lo": dummy})

    nc = _get_nc()
    kwargs = {}
    tdir = os.environ.get("NTX_TRACE_DIR")
    if tdir:
        kwargs = {"trace": True, "tmpdir": tdir, "trace_cores": [0]}
    res = run_bass_kernel_spmd(nc, in_maps, core_ids=list(range(CORES)), **kwargs)
    if tdir:
        _NC_CACHE["last_results"] = res

    # host epilogue (fp64): rowsums, exact diag removal, lse, pos
    zq32 = zq.astype(np.float64)
    diag_logit = 2.0 * (zq32 * zq32).sum(axis=1)  # ||q8(zhat_r)||^2 * 2
    rowsums = np.empty(N, dtype=np.float64)
    for c in range(CORES):
        o = res.results[c]["out"].astype(np.float64)  # [128, 32]
        # rowparts[p, m*4+nb] -> slab row m*128+p
        rs = o.reshape(128, MT, NB).sum(axis=2)  # [128(p), MT]
        for m in range(MT):
            r0 = c * SLAB + m * 128
            rowsums[r0 : r0 + 128] = rs[:, m]
    rowsums -= np.exp(diag_logit)
    lse = np.log(rowsums)

    zh64 = zhat.astype(np.float64)
    pos = 2.0 * np.concatenate(
        [
            (zh64[:B] * zh64[B:]).sum(axis=1),
            (zh64[B:] * zh64[:B]).sum(axis=1),
        ]
    )
    loss = (lse - pos).mean()
    return np.float32(loss)


# revision 4
# speedup vs baseline: 1.5136x; 1.0283x over previous
"""NT-Xent loss (SimCLR) on 8 Trainium2 NeuronCores — V2.

Contract: kernel(z_i, z_j) -> np.float32 scalar loss matching the
reference NT-Xent (temperature 0.5).

Per-core strategy (core c of 8 owns rows [c*1024, (c+1)*1024) of the
8192 x 8192 sim matrix):
  - Host prep ("all-gather the normalized projections" per the sharding
    hint): normalize z rows in fp32, quantize to fp8e4, and ship the
    transposed layout zT [128(D), 8192] that the PE needs — one shared
    array for every core.  Positives and the final log/mean run on host
    (O(N*D) / O(N) work); the O(N^2 * D) matmul + 64M exps stay on
    device.
  - PE: per (m, nb) tile, 4 fp8 matmuls [K=128] x [128, 512] produce a
    [128 slab rows, 2048 cols] fp32 PSUM tile (1 cycle/row at full
    clock; fp8 DoubleRow measured 2x SLOWER than this on HW, so plain
    matmuls are used).
  - The 8.4M exps per core are split across two engines working from a
    shared 2-buffer PSUM ring (8 banks):
      * ACT tiles: exp(2x) via activation scale=2.0 with accum_out ->
        per-row partial sums (1 elem/cycle/lane @1.2GHz).
      * DVE tiles: Schraudolph integer exp2 - pass1 builds bf16 bit
        patterns round((2x*log2e + 127)*128 - c) as i16, pass2
        tensor_reduce sums them as bf16.  Bias constant calibrated so
        the mean ratio to exp() is ~1; residual noise is far inside the
        2e-2 tolerance.
  - Device output: rowparts [128, 32] fp32 per core (one partial row
    sum per (m, nb)).  Host: rowsum per row, subtract the exact
    quantized diagonal exp(2*||q8(zhat_r)||^2), lse = log(.), pos from
    fp32 zhat, loss = mean(lse - pos).
"""

import os
import sys

if "/opt/trn_rl_repo" not in sys.path:
    sys.path.insert(0, "/opt/trn_rl_repo")

import numpy as np
import ml_dtypes

import concourse.bacc as bacc
import concourse.mybir as mybir
import concourse.tile as tile
from concourse.bass_utils import run_bass_kernel_spmd

B = 4096
D = 128
N = 2 * B  # 8192
CORES = 8
SLAB = N // CORES  # 1024 rows per core
MT = SLAB // 128  # 8 m-tiles
NB = N // 2048  # 4 column blocks
EPS = 1e-12

f32 = mybir.dt.float32
bf16 = mybir.dt.bfloat16
fp8 = mybir.dt.float8e4
i16 = mybir.dt.int16

AF = mybir.ActivationFunctionType
OP = mybir.AluOpType

LOG2E = float(np.log2(np.e))
# pass1: bits = round((2x)*log2e*128 + (127*128 - C)) ; bitcast bf16 = exp(2x)
A1 = 2.0 * 128.0 * LOG2E
C_CORR = 0.0574 * 128.0 - 1.204  # empirically debiased (exp1: -0.65% -> ~0)
B1 = 127.0 * 128.0 - C_CORR

# ACT:DVE tile split over the 32 (m, nb) tiles, interleaved evenly.
DVE_TILES = 10


def _consumer_pattern():
    """Return list of 32 bools: True = DVE tile, evenly spread."""
    pat = [False] * 32
    step = 32 / DVE_TILES
    for i in range(DVE_TILES):
        pat[min(31, int(round(i * step + step / 2)))] = True
    # ensure exactly DVE_TILES
    while sum(pat) < DVE_TILES:
        for i in range(32):
            if not pat[i]:
                pat[i] = True
                break
    return pat


def build_nc():
    nc = bacc.Bacc("TRN2", target_bir_lowering=False, debug=False, num_devices=CORES)
    zT = nc.dram_tensor("zT", [128, N], fp8, kind="ExternalInput").ap()
    slab_lo = nc.dram_tensor("slab_lo", [1, 1], f32, kind="ExternalInput").ap()
    out = nc.dram_tensor("out", [128, 32], f32, kind="ExternalOutput").ap()

    pat = _consumer_pattern()

    with tile.TileContext(nc) as tc:
        with (
            tc.tile_pool(name="big", bufs=1) as big,
            tc.tile_pool(name="ring", bufs=2, space="PSUM") as ring,
            tc.tile_pool(name="ebp", bufs=2) as ebp,
        ):
            zt = big.tile([128, N], fp8, tag="zt")
            rowparts = big.tile([128, 32], f32, tag="rowparts")
            wasteA = big.tile([128, 2048], f32, tag="wasteA")
            dummy = big.tile([128, 1], f32, tag="dummy")

            # Columns are host-rotated so this core's slab is first: the
            # first chunk [0:1024] covers every stationary slice plus the
            # first moving halves — issue it first, small, so the main
            # loop starts as early as possible.
            nc.sync.dma_start(zt[:, 0:1024], zT[:, 0:1024])
            nc.sync.dma_start(zt[:, 1024:2048], zT[:, 1024:2048])
            for nb in range(1, NB):
                nc.sync.dma_start(
                    zt[:, nb * 2048 : (nb + 1) * 2048],
                    zT[:, nb * 2048 : (nb + 1) * 2048],
                )

            # warm the exp table set during the DMA phase
            nc.vector.memset(dummy[:], 0.0)
            nc.scalar.activation(
                dummy[:], dummy[:], AF.Exp, bias=0.0, scale=1.0
            )

            def act_tile(m, nb, ps):
                nc.scalar.activation(
                    wasteA[:],
                    ps[:],
                    AF.Exp,
                    bias=0.0,
                    scale=2.0,
                    accum_out=rowparts[:, m * NB + nb : m * NB + nb + 1],
                )

            def dve_tile(m, nb, ps):
                eb = ebp.tile([128, 2048], i16, tag="eb")
                # pass1 split in two so the first PSUM banks release early
                # (lets the PE refill for the next ACT tile ~1.2us sooner)
                nc.vector.tensor_scalar(
                    eb[:, 0:1024], ps[:, 0:1024], A1, B1, OP.mult, OP.add
                )
                nc.vector.tensor_scalar(
                    eb[:, 1024:2048], ps[:, 1024:2048], A1, B1, OP.mult, OP.add
                )
                nc.vector.tensor_reduce(
                    rowparts[:, m * NB + nb : m * NB + nb + 1],
                    eb[:].bitcast(bf16),
                    axis=mybir.AxisListType.X,
                    op=OP.add,
                )

            # slab stationary slices come from zt at per-core offset.
            # SPMD trick: we cannot vary the AP per core, so instead the
            # host passes the slab as a SEPARATE per-core input region by
            # overwriting columns?  No — keep it simple: each core gets
            # its own copy of zT with its slab pre-swapped to the front
            # 1024 columns (host-side cheap).  Stationary = zt[:, 0:1024],
            # moving uses the ORIGINAL column order via a second tensor?
            # That would double DMA.  Instead: host sends zTc where
            # columns are ROTATED so the slab is first: cols' = concat(
            # slab_cols, other_cols).  Row sums are over all columns —
            # permutation-invariant.  Column index j in zTc maps to
            # global column (c*1024 + j) mod 8192, which only matters for
            # the host-side diag correction (row r appears as column
            # (r - c*1024) mod 8192 — never needed on device).
            idx = 0
            for nb in range(NB):
                for m in range(MT):
                    ps = ring.tile([128, 2048], f32, tag="mm")
                    for h in range(4):
                        nc.tensor.matmul(
                            ps[:, h * 512 : (h + 1) * 512],
                            lhsT=zt[:, m * 128 : (m + 1) * 128],
                            rhs=zt[:, nb * 2048 + h * 512 : nb * 2048 + (h + 1) * 512],
                            start=True,
                            stop=True,
                        )
                    if pat[idx]:
                        dve_tile(m, nb, ps)
                    else:
                        act_tile(m, nb, ps)
                    idx += 1

            nc.sync.dma_start(out[:], rowparts[:])

    nc.compile()
    return nc


_NC_CACHE = {}


def _get_nc():
    if "nc" not in _NC_CACHE:
        _NC_CACHE["nc"] = build_nc()
    return _NC_CACHE["nc"]


def kernel(z_i, z_j):
    z_i = np.asarray(z_i, dtype=np.float32)
    z_j = np.asarray(z_j, dtype=np.float32)
    z = np.concatenate([z_i, z_j], axis=0)  # [N, D]
    norm = np.sqrt((z * z).sum(axis=1, keepdims=True))
    zhat = z / np.maximum(norm, EPS)  # fp32 normalized, matches reference

    zq = zhat.astype(ml_dtypes.float8_e4m3)  # TRN fp8e4-compatible (max 240)
    zqT = np.ascontiguousarray(zq.T)  # [128, 8192]

    dummy = np.zeros((1, 1), dtype=np.float32)
    in_maps = []
    for c in range(CORES):
        # rotate columns so this core's slab is first (stationary region)
        rot = np.roll(zqT, -c * SLAB, axis=1)
        in_maps.append({"zT": np.ascontiguousarray(rot), "slab_lo": dummy})

    nc = _get_nc()
    kwargs = {}
    tdir = os.environ.get("NTX_TRACE_DIR")
    if tdir:
        kwargs = {"trace": True, "tmpdir": tdir, "trace_cores": [0]}
    res = run_bass_kernel_spmd(nc, in_maps, core_ids=list(range(CORES)), **kwargs)
    if tdir:
        _NC_CACHE["last_results"] = res

    # host epilogue (fp64): rowsums, exact diag removal, lse, pos
    zq32 = zq.astype(np.float64)
    diag_logit = 2.0 * (zq32 * zq32).sum(axis=1)  # ||q8(zhat_r)||^2 * 2
    rowsums = np.empty(N, dtype=np.float64)
    for c in range(CORES):
        o = res.results[c]["out"].astype(np.float64)  # [128, 32]
        # rowparts[p, m*4+nb] -> slab row m*128+p
        rs = o.reshape(128, MT, NB).sum(axis=2)  # [128(p), MT]
        for m in range(MT):
            r0 = c * SLAB + m * 128
            rowsums[r0 : r0 + 128] = rs[:, m]
    rowsums -= np.exp(diag_logit)
    lse = np.log(rowsums)

    zh64 = zhat.astype(np.float64)
    pos = 2.0 * np.concatenate(
        [
            (zh64[:B] * zh64[B:]).sum(axis=1),
            (zh64[B:] * zh64[:B]).sum(axis=1),
        ]
    )
    loss = (lse - pos).mean()
    return np.float32(loss)


# revision 9
# speedup vs baseline: 1.8064x; 1.1934x over previous
"""NT-Xent loss (SimCLR) on 8 Trainium2 NeuronCores — V3 (symmetric band).

Contract: kernel(z_i, z_j) -> np.float32 scalar loss matching the
reference NT-Xent (temperature 0.5).

The 8192x8192 matrix E = exp(2 zhat zhat^T) is symmetric, so only a
wrapped band needs computing.  Partition the rows into 16 blocks of
512.  Block b computes columns [b*512, b*512 + 4608) (9 chunks of 512,
wrapped mod 8192).  Per row x:
  A_x  = sum of the computed band of row x            (rowsums)
  C_x  = sum over columns x of the computed band rows, EXCLUDING each
         band's first and last 512-chunk              (colsums)
Then A_x covers column-blocks x..x+8, C_x covers x-7..x-1 (by symmetry
E[r,x] = E[x,r]), and since +8 == -8 (mod 16) every one of the 16
column-blocks is covered exactly once.  Row sum = A_x + C_x - E[x,x].

Per-core (core c owns rows [c*1024, (c+1)*1024) = blocks 2c, 2c+1):
  - Host rotates the fp8-quantized zhat^T columns by -c*1024, ships
    [128, 5120] (the union of both blocks' bands).  Host also computes
    positives, the diagonal terms and the final log/mean (O(N*D)).
  - PE: per (m, t) tile [128 rows, 1536 cols], 3 fp8 matmuls; plus
    accumulating ones-matmuls that produce the colsums C directly in
    PSUM (contraction over the 128 partitions).
  - exps split ACT (exp accum_out, writes E to SBUF bf16) / DVE
    (Schraudolph integer exp2 bits -> bf16, tensor_reduce rowsum).
  - Outputs: rowparts [128, 24] and the 8 colsum accumulators [1, 512].
"""

import os
import sys

if "/opt/trn_rl_repo" not in sys.path:
    sys.path.insert(0, "/opt/trn_rl_repo")

import numpy as np
import ml_dtypes

import concourse.bacc as bacc
import concourse.mybir as mybir
import concourse.tile as tile
from concourse.bass_utils import run_bass_kernel_spmd

B = 4096
D = 128
N = 2 * B  # 8192
CORES = 8
SLAB = N // CORES  # 1024 rows per core
MT = 8  # m-tiles of 128 rows
TW = 1536  # tile width
NT = 3  # tiles per m (band 4608 = 3*1536)
BAND = 4608
LOAD = 5120  # rotated columns loaded (union of both 512-blocks' bands)
EPS = 1e-12

f32 = mybir.dt.float32
bf16 = mybir.dt.bfloat16
fp8 = mybir.dt.float8e4
i16 = mybir.dt.int16

AF = mybir.ActivationFunctionType
OP = mybir.AluOpType

LOG2E = float(np.log2(np.e))
A1 = 2.0 * 128.0 * LOG2E
C_CORR = 0.0574 * 128.0 - 1.204
B1 = 127.0 * 128.0 - C_CORR

# DVE-consumer positions (t, m); 7 of 24, spread to avoid adjacency
DVE_SET = {(0, 1), (0, 4), (1, 2), (1, 5), (2, 0), (2, 3), (2, 7)}


def _slot_contribs():
    """Emission-ordered list of colsum contributions per slot s (1..8).

    Tile (t, m) covers band-relative chunks g = 3t + k (k in 0..2).
    base = 0 for m<4 (block B0), 512 for m>=4 (B1).  Rotated chunk
    s = base/512 + g.  Colsums include only band-relative g in [1, 7].
    Returns {s: [(t, m, k), ...]} in emission order (t-major, m inner).
    """
    out = {s: [] for s in range(1, 9)}
    for t in range(NT):
        for m in range(MT):
            half = 0 if m < 4 else 1
            for k in range(3):
                g = 3 * t + k
                if 1 <= g <= 7:
                    s = half + g
                    out[s].append((t, m, k))
    return out


def build_nc():
    nc = bacc.Bacc("TRN2", target_bir_lowering=False, debug=False, num_devices=CORES)
    zT = nc.dram_tensor("zT", [128, LOAD], fp8, kind="ExternalInput").ap()
    out = nc.dram_tensor("out", [128, 24], f32, kind="ExternalOutput").ap()
    cout = nc.dram_tensor("cout", [8, 512], f32, kind="ExternalOutput").ap()

    contribs = _slot_contribs()
    first = {}
    last = {}
    for s, lst in contribs.items():
        first[s] = lst[0]
        last[s] = lst[-1]

    with tile.TileContext(nc) as tc:
        with (
            tc.tile_pool(name="big", bufs=1) as big,
            tc.tile_pool(name="ring", bufs=2, space="PSUM") as ring,
            tc.tile_pool(name="accp", bufs=1, space="PSUM") as accp,
            tc.tile_pool(name="ebp", bufs=2) as ebp,
            tc.tile_pool(name="wap", bufs=2) as wap,
        ):
            zt = big.tile([128, LOAD], fp8, tag="zt")
            rowparts = big.tile([128, 24], f32, tag="rowparts")
            csb = big.tile([128, 1024], f32, tag="csb")  # staged colsums
            ones = big.tile([128, 1], bf16, tag="ones")
            dummy = big.tile([128, 1], f32, tag="dummy")

            nc.sync.dma_start(zt[:, 0:1536], zT[:, 0:1536])
            nc.sync.dma_start(zt[:, 1536:3072], zT[:, 1536:3072])
            nc.sync.dma_start(zt[:, 3072:4608], zT[:, 3072:4608])
            nc.sync.dma_start(zt[:, 4608:5120], zT[:, 4608:5120])

            nc.vector.memset(ones[:], 1.0)
            nc.vector.memset(dummy[:], 0.0)
            nc.scalar.activation(dummy[:], dummy[:], AF.Exp, bias=0.0, scale=1.0)

            acc = accp.tile([128, 1024], f32, tag="acc")  # 2 banks, 8 slots

            def acc_view(s):
                i = s - 1
                boff = (i // 4) * 512
                p = 32 * (i % 4)
                return acc[p : p + 1, boff : boff + 512]

            def colsum(esb, t, m, k):
                half = 0 if m < 4 else 1
                g = 3 * t + k
                if not (1 <= g <= 7):
                    return
                s = half + g
                is_last = last[s] == (t, m, k)
                nc.tensor.matmul(
                    acc_view(s),
                    lhsT=ones[:],
                    rhs=esb[:, k * 512 : (k + 1) * 512],
                    start=(first[s] == (t, m, k)),
                    stop=is_last,
                    skip_group_check=True,
                    tile_position=(0, 32 * ((s - 1) % 4)),
                )
                if is_last:
                    i = s - 1
                    boff = (i // 4) * 512
                    p = 32 * (i % 4)
                    cs = csb[p : p + 1, boff : boff + 512]
                    nc.vector.tensor_copy(cs, acc_view(s))
                    nc.sync.dma_start(cout[s - 1 : s, :], cs)

            for t in range(NT):
                for m in range(MT):
                    base = 0 if m < 4 else 512
                    col0 = base + t * TW
                    ps = ring.tile([128, TW], f32, tag="mm")
                    for h in range(3):
                        nc.tensor.matmul(
                            ps[:, h * 512 : (h + 1) * 512],
                            lhsT=zt[:, m * 128 : (m + 1) * 128],
                            rhs=zt[:, col0 + h * 512 : col0 + (h + 1) * 512],
                            start=True,
                            stop=True,
                        )
                    rp = rowparts[:, t * MT + m : t * MT + m + 1]
                    if (t, m) in DVE_SET:
                        eb = ebp.tile([128, TW], i16, tag="eb")
                        nc.vector.tensor_scalar(
                            eb[:, 0:1024], ps[:, 0:1024], A1, B1, OP.mult, OP.add
                        )
                        nc.vector.tensor_scalar(
                            eb[:, 1024:TW], ps[:, 1024:TW], A1, B1, OP.mult, OP.add
                        )
                        ebf = eb[:].bitcast(bf16)
                        nc.vector.tensor_reduce(
                            rp, ebf, axis=mybir.AxisListType.X, op=OP.add
                        )
                        for k in range(3):
                            colsum(ebf, t, m, k)
                    else:
                        wa = wap.tile([128, TW], bf16, tag="wa")
                        nc.scalar.activation(
                            wa[:], ps[:], AF.Exp, bias=0.0, scale=2.0, accum_out=rp
                        )
                        for k in range(3):
                            colsum(wa[:], t, m, k)

            nc.sync.dma_start(out[:], rowparts[:])

    nc.compile()
    return nc


_NC_CACHE = {}


def _get_nc():
    if "nc" not in _NC_CACHE:
        _NC_CACHE["nc"] = build_nc()
    return _NC_CACHE["nc"]


def kernel(z_i, z_j):
    z_i = np.asarray(z_i, dtype=np.float32)
    z_j = np.asarray(z_j, dtype=np.float32)
    z = np.concatenate([z_i, z_j], axis=0)  # [N, D]
    norm = np.sqrt((z * z).sum(axis=1, keepdims=True))
    zhat = z / np.maximum(norm, EPS)

    zq = zhat.astype(ml_dtypes.float8_e4m3)
    zqT = np.ascontiguousarray(zq.T)  # [128, 8192]

    in_maps = []
    for c in range(CORES):
        rot = np.roll(zqT, -c * SLAB, axis=1)[:, :LOAD]
        in_maps.append({"zT": np.ascontiguousarray(rot)})

    nc = _get_nc()
    kwargs = {}
    tdir = os.environ.get("NTX_TRACE_DIR")
    if tdir:
        kwargs = {"trace": True, "tmpdir": tdir, "trace_cores": [0]}
    res = run_bass_kernel_spmd(nc, in_maps, core_ids=list(range(CORES)), **kwargs)
    if tdir:
        _NC_CACHE["last_results"] = res

    # host epilogue in fp64
    A = np.zeros(N, dtype=np.float64)
    C = np.zeros(N, dtype=np.float64)
    for c in range(CORES):
        o = res.results[c]["out"].astype(np.float64)  # [128, 24]
        rs = o.reshape(128, NT, MT).sum(axis=1)  # [p, m] (cols t*MT+m)
        for m in range(MT):
            r0 = c * SLAB + m * 128
            A[r0 : r0 + 128] += rs[:, m]
        co = res.results[c]["cout"].astype(np.float64)  # [8, 512] slots 1..8
        for s in range(1, 9):
            j0 = (c * SLAB + s * 512) % N
            C[j0 : j0 + 512] += co[s - 1]

    zq64 = zq.astype(np.float64)
    diag = np.exp(2.0 * (zq64 * zq64).sum(axis=1))
    rowsums = A + C - diag
    lse = np.log(rowsums)

    zh64 = zhat.astype(np.float64)
    pos = 2.0 * np.concatenate(
        [
            (zh64[:B] * zh64[B:]).sum(axis=1),
            (zh64[B:] * zh64[:B]).sum(axis=1),
        ]
    )
    loss = (lse - pos).mean()
    return np.float32(loss)


# revision 10
# speedup vs baseline: 1.9472x; 1.0780x over previous
"""NT-Xent loss (SimCLR) on 8 Trainium2 NeuronCores — V3 (symmetric band).

Contract: kernel(z_i, z_j) -> np.float32 scalar loss matching the
reference NT-Xent (temperature 0.5).

The 8192x8192 matrix E = exp(2 zhat zhat^T) is symmetric, so only a
wrapped band needs computing.  Partition the rows into 16 blocks of
512.  Block b computes columns [b*512, b*512 + 4608) (9 chunks of 512,
wrapped mod 8192).  Per row x:
  A_x  = sum of the computed band of row x            (rowsums)
  C_x  = sum over columns x of the computed band rows, EXCLUDING each
         band's first and last 512-chunk              (colsums)
Then A_x covers column-blocks x..x+8, C_x covers x-7..x-1 (by symmetry
E[r,x] = E[x,r]), and since +8 == -8 (mod 16) every one of the 16
column-blocks is covered exactly once.  Row sum = A_x + C_x - E[x,x].

Per-core (core c owns rows [c*1024, (c+1)*1024) = blocks 2c, 2c+1):
  - Host rotates the fp8-quantized zhat^T columns by -c*1024, ships
    [128, 5120] (the union of both blocks' bands).  Host also computes
    positives, the diagonal terms and the final log/mean (O(N*D)).
  - PE: per (m, t) tile [128 rows, 1536 cols], 3 fp8 matmuls; plus
    accumulating ones-matmuls that produce the colsums C directly in
    PSUM (contraction over the 128 partitions).
  - exps split ACT (exp accum_out, writes E to SBUF bf16) / DVE
    (Schraudolph integer exp2 bits -> bf16, tensor_reduce rowsum).
  - Outputs: rowparts [128, 24] and the 8 colsum accumulators [1, 512].
"""

import os
import sys

if "/opt/trn_rl_repo" not in sys.path:
    sys.path.insert(0, "/opt/trn_rl_repo")

import numpy as np
import ml_dtypes

import concourse.bacc as bacc
import concourse.mybir as mybir
import concourse.tile as tile
from concourse.bass_utils import run_bass_kernel_spmd

B = 4096
D = 128
N = 2 * B  # 8192
CORES = 8
SLAB = N // CORES  # 1024 rows per core
MT = 8  # m-tiles of 128 rows
TW = 1536  # tile width
NT = 3  # tiles per m (band 4608 = 3*1536)
BAND = 4608
LOAD = 5120  # rotated columns loaded (union of both 512-blocks' bands)
EPS = 1e-12

f32 = mybir.dt.float32
bf16 = mybir.dt.bfloat16
fp8 = mybir.dt.float8e4
i16 = mybir.dt.int16

AF = mybir.ActivationFunctionType
OP = mybir.AluOpType

LOG2E = float(np.log2(np.e))
A1 = 2.0 * 128.0 * LOG2E
C_CORR = 0.0574 * 128.0 - 1.204
B1 = 127.0 * 128.0 - C_CORR

# DVE-consumer positions (t, m); 7 of 24, spread to avoid adjacency
DVE_SET = {(0, 1), (0, 4), (1, 2), (1, 5), (2, 0), (2, 3), (2, 7)}


def _slot_contribs():
    """Emission-ordered list of colsum contributions per slot s (1..8).

    Tile (t, m) covers band-relative chunks g = 3t + k (k in 0..2).
    base = 0 for m<4 (block B0), 512 for m>=4 (B1).  Rotated chunk
    s = base/512 + g.  Colsums include only band-relative g in [1, 7].
    Returns {s: [(t, m, k), ...]} in emission order (t-major, m inner).
    """
    out = {s: [] for s in range(1, 9)}
    for t in range(NT):
        for m in range(MT):
            half = 0 if m < 4 else 1
            for k in range(3):
                g = 3 * t + k
                if 1 <= g <= 7:
                    s = half + g
                    out[s].append((t, m, k))
    return out


def build_nc():
    nc = bacc.Bacc("TRN2", target_bir_lowering=False, debug=False, num_devices=CORES)
    zT = nc.dram_tensor("zT", [128, LOAD], fp8, kind="ExternalInput").ap()
    out = nc.dram_tensor("out", [128, 24], f32, kind="ExternalOutput").ap()
    cout = nc.dram_tensor("cout", [8, 512], f32, kind="ExternalOutput").ap()

    contribs = _slot_contribs()
    first = {}
    last = {}
    for s, lst in contribs.items():
        first[s] = lst[0]
        last[s] = lst[-1]

    with tile.TileContext(nc) as tc:
        with (
            tc.tile_pool(name="big", bufs=1) as big,
            tc.tile_pool(name="ring", bufs=2, space="PSUM") as ring,
            tc.tile_pool(name="accp", bufs=1, space="PSUM") as accp,
            tc.tile_pool(name="ebp", bufs=2) as ebp,
            tc.tile_pool(name="wap", bufs=2) as wap,
        ):
            zt = big.tile([128, LOAD], fp8, tag="zt")
            rowparts = big.tile([128, 24], f32, tag="rowparts")
            csb = big.tile([128, 1024], f32, tag="csb")  # staged colsums
            ones = big.tile([128, 1], bf16, tag="ones")
            dummy = big.tile([128, 1], f32, tag="dummy")

            nc.sync.dma_start(zt[:, 0:1536], zT[:, 0:1536])
            nc.sync.dma_start(zt[:, 1536:3072], zT[:, 1536:3072])
            nc.sync.dma_start(zt[:, 3072:4608], zT[:, 3072:4608])
            nc.sync.dma_start(zt[:, 4608:5120], zT[:, 4608:5120])

            nc.vector.memset(ones[:], 1.0)
            nc.vector.memset(dummy[:], 0.0)
            nc.scalar.activation(dummy[:], dummy[:], AF.Exp, bias=0.0, scale=1.0)

            acc = accp.tile([128, 1024], f32, tag="acc")  # 2 banks, 8 slots

            def acc_view(s):
                i = s - 1
                boff = (i // 4) * 512
                p = 32 * (i % 4)
                return acc[p : p + 1, boff : boff + 512]

            def colsum(esb, t, m, k):
                half = 0 if m < 4 else 1
                g = 3 * t + k
                if not (1 <= g <= 7):
                    return
                s = half + g
                is_last = last[s] == (t, m, k)
                nc.tensor.matmul(
                    acc_view(s),
                    lhsT=ones[:],
                    rhs=esb[:, k * 512 : (k + 1) * 512],
                    start=(first[s] == (t, m, k)),
                    stop=is_last,
                    skip_group_check=True,
                    tile_position=(0, 32 * ((s - 1) % 4)),
                )
                if is_last:
                    i = s - 1
                    boff = (i // 4) * 512
                    p = 32 * (i % 4)
                    cs = csb[p : p + 1, boff : boff + 512]
                    nc.vector.tensor_copy(cs, acc_view(s))
                    nc.sync.dma_start(cout[s - 1 : s, :], cs)

            # Software-pipelined emission: the colsum matmuls of tile
            # i-1 are emitted after tile i's production, so the in-order
            # PE stream never waits on a consumer before producing the
            # next tile.
            pending = None  # (esb, t, m) whose colsums are not yet emitted
            for t in range(NT):
                for m in range(MT):
                    base = 0 if m < 4 else 512
                    col0 = base + t * TW
                    ps = ring.tile([128, TW], f32, tag="mm")
                    for h in range(3):
                        nc.tensor.matmul(
                            ps[:, h * 512 : (h + 1) * 512],
                            lhsT=zt[:, m * 128 : (m + 1) * 128],
                            rhs=zt[:, col0 + h * 512 : col0 + (h + 1) * 512],
                            start=True,
                            stop=True,
                        )
                    if pending is not None:
                        pe, pt, pm = pending
                        for k in range(3):
                            colsum(pe, pt, pm, k)
                    rp = rowparts[:, t * MT + m : t * MT + m + 1]
                    if (t, m) in DVE_SET:
                        eb = ebp.tile([128, TW], i16, tag="eb")
                        nc.vector.tensor_scalar(
                            eb[:, 0:1024], ps[:, 0:1024], A1, B1, OP.mult, OP.add
                        )
                        nc.vector.tensor_scalar(
                            eb[:, 1024:TW], ps[:, 1024:TW], A1, B1, OP.mult, OP.add
                        )
                        ebf = eb[:].bitcast(bf16)
                        nc.vector.tensor_reduce(
                            rp, ebf, axis=mybir.AxisListType.X, op=OP.add
                        )
                        pending = (ebf, t, m)
                    else:
                        wa = wap.tile([128, TW], bf16, tag="wa")
                        nc.scalar.activation(
                            wa[:], ps[:], AF.Exp, bias=0.0, scale=2.0, accum_out=rp
                        )
                        pending = (wa[:], t, m)
            pe, pt, pm = pending
            for k in range(3):
                colsum(pe, pt, pm, k)

            nc.sync.dma_start(out[:], rowparts[:])

    nc.compile()
    return nc


_NC_CACHE = {}


def _get_nc():
    if "nc" not in _NC_CACHE:
        _NC_CACHE["nc"] = build_nc()
    return _NC_CACHE["nc"]


def kernel(z_i, z_j):
    z_i = np.asarray(z_i, dtype=np.float32)
    z_j = np.asarray(z_j, dtype=np.float32)
    z = np.concatenate([z_i, z_j], axis=0)  # [N, D]
    norm = np.sqrt((z * z).sum(axis=1, keepdims=True))
    zhat = z / np.maximum(norm, EPS)

    zq = zhat.astype(ml_dtypes.float8_e4m3)
    zqT = np.ascontiguousarray(zq.T)  # [128, 8192]

    in_maps = []
    for c in range(CORES):
        rot = np.roll(zqT, -c * SLAB, axis=1)[:, :LOAD]
        in_maps.append({"zT": np.ascontiguousarray(rot)})

    nc = _get_nc()
    kwargs = {}
    tdir = os.environ.get("NTX_TRACE_DIR")
    if tdir:
        kwargs = {"trace": True, "tmpdir": tdir, "trace_cores": [0]}
    res = run_bass_kernel_spmd(nc, in_maps, core_ids=list(range(CORES)), **kwargs)
    if tdir:
        _NC_CACHE["last_results"] = res

    # host epilogue in fp64
    A = np.zeros(N, dtype=np.float64)
    C = np.zeros(N, dtype=np.float64)
    for c in range(CORES):
        o = res.results[c]["out"].astype(np.float64)  # [128, 24]
        rs = o.reshape(128, NT, MT).sum(axis=1)  # [p, m] (cols t*MT+m)
        for m in range(MT):
            r0 = c * SLAB + m * 128
            A[r0 : r0 + 128] += rs[:, m]
        co = res.results[c]["cout"].astype(np.float64)  # [8, 512] slots 1..8
        for s in range(1, 9):
            j0 = (c * SLAB + s * 512) % N
            C[j0 : j0 + 512] += co[s - 1]

    zq64 = zq.astype(np.float64)
    diag = np.exp(2.0 * (zq64 * zq64).sum(axis=1))
    rowsums = A + C - diag
    lse = np.log(rowsums)

    zh64 = zhat.astype(np.float64)
    pos = 2.0 * np.concatenate(
        [
            (zh64[:B] * zh64[B:]).sum(axis=1),
            (zh64[B:] * zh64[:B]).sum(axis=1),
        ]
    )
    loss = (lse - pos).mean()
    return np.float32(loss)
